# revision 1
# baseline (speedup 1.0000x reference)
"""Multi-head cross attention on 8 Trainium2 NeuronCores.

Sharding: core c = b*4 + g handles batch b (of 2) and head-group g (4 heads
of the 16).  Each core projects Q/K/V for its 4 heads, runs attention, and
computes a partial output projection with its 256 rows of Wo; the host sums
the 4 partials per batch (plus bo and the bv@Wo term, exact because softmax
rows sum to 1).

Dataflow is fully "transposed" so no on-device transposes are needed:
  - host passes x^T in bf16 (transposed + cast on CPU)
  - QT[dh, sq]  = Wq_g.T @ xqT       (lhsT = Wq slice, rhs = xqT)
  - KT[dh, skv] = Wk_g.T @ xkvT      (two heads packed per 128-partition tile)
  - V[skv, dh]  = xkvT.T @ Wv'_g     (lhsT = xkvT slice, rhs = Wv' which has
                                      a zero column after each head, turned
                                      into ones -> fused softmax row-sums)
  - S^T[skv, q] = KT_h.T @ QT_h      (K=64; the two heads of a pair use
                                      disjoint PE row groups and execute
                                      concurrently; both land in one 1024-wide
                                      PSUM tile so one ACT Exp covers both)
  - P^T = exp(S^T / 8)               (no max subtraction; |scores/8| < ~3)
  - O'^T = [V_h|1|...].T @ P^T       (lhsT window is 128 wide so the PE array
                                      is fully used and the HAM clock gate
                                      stays at 2.4 GHz; rows 65-127 are junk,
                                      row 64 is the softmax row-sum)
  - O^T = O'^T[0:64] * (1/rowsum)    (reciprocal on DVE, broadcast across
                                      partitions on the idle GpSimd engine,
                                      multiply on DVE -- nothing touches the
                                      PE queue or PSUM, so block boundaries
                                      don't stall the matmul pipeline)
  - out_partial[sq, 1024] = O^T_allheads.T @ Wo_g
Attention runs in 8 blocks (head-pair x query-quarter); each block's
normalize is emitted two kv-tiles into the next block so its instructions
sit behind fresh matmul work in every engine queue.
Matmuls run in bf16 (fp32 PSUM accumulation; measured rel err ~4e-3).
fp32r (1 cyc/row in the cost model) measured ~3.5 cyc/row on HW and is
throttled in exactly the shapes attention needs, so bf16 wins.
"""

import sys

sys.path.insert(0, "/opt/trn_rl_repo")

import ml_dtypes
import numpy as np

BF16NP = ml_dtypes.bfloat16

B, SQ, SKV, D, H = 2, 2048, 2048, 1024, 16
DH = D // H          # 64
N_CORES = 8
G = 4                # head groups
HPG = H // G         # heads per group = 4
GC = HPG * DH        # group width = 256

_nc_cache = None


def _build_nc():
    import concourse.mybir as mybir
    import concourse.tile as tile
    from concourse import bacc

    F32 = mybir.dt.float32
    F32R = mybir.dt.float32r
    BF16 = mybir.dt.bfloat16
    AF = mybir.ActivationFunctionType
    MUL = mybir.AluOpType.mult

    nc = bacc.Bacc("TRN2", target_bir_lowering=False, debug=False,
                   num_devices=N_CORES)

    xqT_d = nc.dram_tensor("xqT", [D, SQ], BF16, kind="ExternalInput").ap()
    xkvT_d = nc.dram_tensor("xkvT", [D, SKV], BF16, kind="ExternalInput").ap()
    wq_d = nc.dram_tensor("wq", [D, GC], BF16, kind="ExternalInput").ap()
    wk_d = nc.dram_tensor("wk", [D, GC], BF16, kind="ExternalInput").ap()
    # Wv' with a zero column after each head's 64 (slots for the ones column)
    wvp_d = nc.dram_tensor("wvp", [D, HPG * 65], BF16, kind="ExternalInput").ap()
    wo_d = nc.dram_tensor("wo", [GC, D], BF16, kind="ExternalInput").ap()
    bq_d = nc.dram_tensor("bq2", [128, 2], F32, kind="ExternalInput").ap()
    bk_d = nc.dram_tensor("bk2", [128, 2], F32, kind="ExternalInput").ap()
    ones_d = nc.dram_tensor("ones64", [1, 128], F32R, kind="ExternalInput").ap()
    out_d = nc.dram_tensor("out_p", [SQ, D], F32, kind="ExternalOutput").ap()

    ND = D // 128        # 8 d-tiles (contraction over D)
    NJ = SKV // 128      # 16 kv tiles
    VW = HPG * 65        # 260, V' row width
    scale = 1.0 / float(np.sqrt(DH))

    with tile.TileContext(nc) as tc:
        with (
            tc.tile_pool(name="persist", bufs=1) as pp,
            tc.tile_pool(name="pha", bufs=1) as pa,
            tc.tile_pool(name="phb", bufs=1) as pb,
        ):
            # ---- persistent tiles -------------------------------------
            qt_sb = pp.tile([128, 2 * SQ], BF16, tag="qt_sb")
            kt_sb = pp.tile([128, 2 * SKV], BF16, tag="kt_sb")
            vp_sb = pp.tile([128, NJ * VW + 63], BF16, tag="vp_sb")
            o_sbA = pp.tile([128, 2 * 1024], BF16, tag="o_sbA")
            o_sbB = pp.tile([128, 2 * 1024], BF16, tag="o_sbB")
            bq_sb = pp.tile([128, 2], F32, tag="bq_sb")
            bk_sb = pp.tile([128, 2], F32, tag="bk_sb")
            ones_sb = pp.tile([1, 128], F32R, tag="ones_sb")
            nc.sync.dma_start(out=bq_sb[:], in_=bq_d[:])
            nc.sync.dma_start(out=bk_sb[:], in_=bk_d[:])
            nc.sync.dma_start(out=ones_sb[:], in_=ones_d[:])

            # ---- phase A: load xkvT, weights; compute KT and V' -------
            wk_sb = pa.tile([128, ND * GC], BF16, tag="wk_sb")
            wvp_sb = pa.tile([128, ND * VW], BF16, tag="wvp_sb")
            for d in range(ND):
                nc.sync.dma_start(
                    out=wk_sb[:, d * GC:(d + 1) * GC],
                    in_=wk_d[d * 128:(d + 1) * 128, :])
            xkv = []
            for d in range(ND):
                t = pa.tile([128, SKV], BF16, tag=f"xkv{d}", name=f"xkv{d}")
                nc.gpsimd.dma_start(out=t[:], in_=xkvT_d[d * 128:(d + 1) * 128, :])
                xkv.append(t)

            with tc.tile_pool(name="psA", bufs=1, space="PSUM") as psA:
                # KT (2 pair-tiles x 4 q chunks); d-outer so each xkv DMA
                # tile is consumed as soon as it lands
                pk = {}
                for p in range(2):
                    for qc in range(4):
                        pk[p, qc] = psA.tile([128, 512], F32, tag="pk",
                                             bufs=8, name=f"pk{p}{qc}")
                for d in range(ND):
                    for p in range(2):
                        for qc in range(4):
                            nc.tensor.matmul(
                                pk[p, qc][:],
                                wk_sb[:, d * GC + p * 128:d * GC + (p + 1) * 128],
                                xkv[d][:, qc * 512:(qc + 1) * 512],
                                start=(d == 0), stop=(d == ND - 1),
                            )
                for p in range(2):
                    for qc in range(4):
                        nc.scalar.activation(
                            kt_sb[:, p * SKV + qc * 512:p * SKV + (qc + 1) * 512],
                            pk[p, qc][:], AF.Identity, bias=bk_sb[:, p:p + 1])
                # V' (16 kv tiles, accumulate over d)
                for d in range(ND):
                    nc.sync.dma_start(
                        out=wvp_sb[:, d * VW:(d + 1) * VW],
                        in_=wvp_d[d * 128:(d + 1) * 128, :])
                for j in range(NJ):
                    pv = psA.tile([128, VW], F32, tag="pk", bufs=8,
                                  name=f"pv{j}")
                    for d in range(ND):
                        nc.tensor.matmul(
                            pv[:],
                            xkv[d][:, j * 128:(j + 1) * 128],
                            wvp_sb[:, d * VW:(d + 1) * VW],
                            start=(d == 0), stop=(d == ND - 1),
                        )
                    nc.vector.tensor_copy(vp_sb[:, j * VW:(j + 1) * VW], pv[:])
                # ones columns of V' (stride-65 view hits col 64 of each head)
                oc = vp_sb[:, 64:NJ * VW:65]
                nc.scalar.activation(oc, oc, AF.Copy, scale=0.0, bias=1.0)
                # zero tail pad (scale-0 copy from finite psum keeps NaNs out)
                nc.scalar.activation(vp_sb[:, NJ * VW:NJ * VW + 63],
                                     pv[:, 0:63], AF.Copy, scale=0.0)

                # ---- phase B: stream xqT, compute QT ------------------
                wq_sb = pb.tile([128, ND * GC], BF16, tag="wq_sb")
                for d in range(ND):
                    nc.sync.dma_start(
                        out=wq_sb[:, d * GC:(d + 1) * GC],
                        in_=wq_d[d * 128:(d + 1) * 128, :])
                xq_tiles = []
                for d in range(ND):
                    xq_t = pb.tile([128, SQ], BF16, tag="xq", bufs=3,
                                   name=f"xq{d}")
                    nc.gpsimd.dma_start(out=xq_t[:],
                                        in_=xqT_d[d * 128:(d + 1) * 128, :])
                    xq_tiles.append(xq_t)
                pq = {}
                for p in range(2):
                    for qc in range(4):
                        pq[p, qc] = psA.tile([128, 512], F32, tag="pk", bufs=8,
                                             name=f"pq{p}{qc}")
                for d in range(ND):
                    xq_t = xq_tiles[d]
                    for p in range(2):
                        for qc in range(4):
                            nc.tensor.matmul(
                                pq[p, qc][:],
                                wq_sb[:, d * GC + p * 128:d * GC + (p + 1) * 128],
                                xq_t[:, qc * 512:(qc + 1) * 512],
                                start=(d == 0), stop=(d == ND - 1),
                            )
                for p in range(2):
                    for qc in range(4):
                        blk = slice(p * SQ + qc * 512, p * SQ + (qc + 1) * 512)
                        nc.scalar.activation(
                            qt_sb[:, blk], pq[p, qc][:],
                            AF.Identity, bias=bq_sb[:, p:p + 1])

            # ---- attention -------------------------------------------
            with (
                tc.tile_pool(name="attn", bufs=1) as at,
                tc.tile_pool(name="psC", bufs=1, space="PSUM") as psC,
                tc.tile_pool(name="oproj", bufs=1) as op_pool,
                tc.tile_pool(name="psD", bufs=1, space="PSUM") as psD,
            ):
                wo_sb = op_pool.tile([128, 2 * D], BF16, tag="wo_sb")
                nc.sync.dma_start(
                    out=wo_sb[:].rearrange("p (t n) -> p t n", t=2),
                    in_=wo_d.rearrange("(t p) n -> p t n", p=128),
                )

                def emit_outproj(lo, hi):
                    for s in range(lo, hi):
                        for n2 in range(2):
                            po = psD.tile([128, 512], F32, tag="po", bufs=2,
                                          name=f"po{s}{n2}")
                            o_half = o_sbA if s < 8 else o_sbB
                            s8 = s % 8
                            for tt in range(2):
                                nc.tensor.matmul(
                                    po[:],
                                    o_half[:, tt * 1024 + s8 * 128:
                                           tt * 1024 + (s8 + 1) * 128],
                                    wo_sb[:, tt * D + n2 * 512:
                                          tt * D + n2 * 512 + 512],
                                    start=(tt == 0), stop=(tt == 1),
                                )
                            ob = op_pool.tile([128, 512], F32, tag="ob",
                                              bufs=3, name=f"ob{s}{n2}")
                            nc.vector.tensor_copy(ob[:], po[:])
                            nc.sync.dma_start(
                                out=out_d[s * 128:(s + 1) * 128,
                                          n2 * 512:(n2 + 1) * 512],
                                in_=ob[:])

                pending_norm = []

                def flush_norm():
                    while pending_norm:
                        pending_norm.pop(0)()

                for t in range(2):          # head pair
                    for qq in range(4):     # q quarter (512)
                        o_ps = {}
                        for hp in range(2):
                            o_ps[hp] = psC.tile(
                                [128, 512], F32, tag="o_ps", bufs=2,
                                name=f"o_ps{t}{qq}{hp}")
                        for j in range(NJ):
                            st = psC.tile([128, 1024], F32, tag="st2", bufs=2,
                                          name=f"st{t}{qq}{j}")
                            # K=64 scores; the two heads use disjoint row
                            # groups (partitions 0-63 / 64-127) and execute
                            # concurrently on the PE
                            for hp in range(2):
                                nc.tensor.matmul(
                                    st[:, hp * 512:(hp + 1) * 512],
                                    kt_sb[hp * 64:(hp + 1) * 64,
                                          t * SKV + j * 128:
                                          t * SKV + (j + 1) * 128],
                                    qt_sb[hp * 64:(hp + 1) * 64,
                                          t * SQ + qq * 512:
                                          t * SQ + (qq + 1) * 512],
                                    start=True, stop=True,
                                )
                            p_t = at.tile([128, 1024], BF16, tag="pt",
                                          bufs=6, name=f"pt{t}{qq}{j}")
                            nc.scalar.activation(p_t[:], st[:],
                                                 AF.Exp, scale=scale)
                            for hp in range(2):
                                h = 2 * t + hp
                                nc.tensor.matmul(
                                    o_ps[hp][:],
                                    vp_sb[:, j * VW + h * 65:
                                          j * VW + h * 65 + 128],
                                    p_t[:, hp * 512:(hp + 1) * 512],
                                    start=(j == 0), stop=(j == NJ - 1),
                                )
                            if j == 1 and t == 1 and qq == 3:
                                # o_sbA's last normalize (t1,qq1) is already
                                # emitted; its outproj half can gap-fill the
                                # PE during the final attention blocks
                                flush_norm()
                                emit_outproj(0, 8)
                            elif j == 1:
                                # emit the previous block's normalize now --
                                # its bc matmuls land behind this block's
                                # first scores in the PE stream, so the PE
                                # never head-of-line blocks on the slow
                                # reciprocal chain
                                flush_norm()
                        # stage rowsums out of PSUM quickly, then queue the
                        # rest of the normalize for later emission
                        for hp in range(2):
                            ot = at.tile([64, 512], F32, tag="ot",
                                         bufs=4, name=f"ot{t}{qq}{hp}")
                            nc.vector.tensor_copy(ot[:], o_ps[hp][0:64, :])
                            rs = at.tile([1, 512], F32, tag="rs", bufs=4,
                                         name=f"rs{t}{qq}{hp}")
                            nc.vector.tensor_copy(rs[:], o_ps[hp][64:65, :])

                            def norm(t=t, qq=qq, hp=hp, ot=ot, rs=rs):
                                rcp = at.tile([1, 512], F32, tag="rcp",
                                              bufs=4, name=f"rcp{t}{qq}{hp}")
                                nc.vector.reciprocal(rcp[:], rs[:])
                                bcs = at.tile([64, 512], F32, tag="bcs",
                                              bufs=4, name=f"bcs{t}{qq}{hp}")
                                nc.gpsimd.partition_broadcast(
                                    bcs[:], rcp[:], channels=64)
                                o_half = o_sbA if qq < 2 else o_sbB
                                col = t * 1024 + (qq % 2) * 512
                                nc.vector.tensor_tensor(
                                    out=o_half[hp * 64:(hp + 1) * 64,
                                               col:col + 512],
                                    in0=ot[:], in1=bcs[:],
                                    op=MUL)

                            pending_norm.append(norm)
                flush_norm()

                # ---- output projection (second half; first half was
                # emitted inside the attention loop) ------------------------
                emit_outproj(8, 16)

    nc.compile()
    return nc


def build_in_maps(inputs):
    query_input = np.asarray(inputs["query_input"], dtype=np.float32)
    kv_input = np.asarray(inputs["kv_input"], dtype=np.float32)
    Wq = np.asarray(inputs["Wq"], dtype=np.float32)
    bq = np.asarray(inputs["bq"], dtype=np.float32)
    Wkv = np.asarray(inputs["Wkv"], dtype=np.float32)
    bkv = np.asarray(inputs["bkv"], dtype=np.float32)
    Wo = np.asarray(inputs["Wo"], dtype=np.float32)

    Wk = Wkv[:, :D]
    Wv = Wkv[:, D:]
    bk = bkv[:D]
    ones64 = np.ones((1, 128), np.float32)

    xT = [np.ascontiguousarray(query_input[b].T).astype(BF16NP) for b in range(B)]
    kvT = [np.ascontiguousarray(kv_input[b].T).astype(BF16NP) for b in range(B)]

    in_maps = []
    for c in range(N_CORES):
        b, g = divmod(c, G)
        c0 = g * GC
        wvp = np.zeros((D, HPG * 65), np.float32)
        for h in range(HPG):
                wvp[:, h * 65:h * 65 + 64] = Wv[:, c0 + h * DH:c0 + (h + 1) * DH]
        bq2 = bq[c0:c0 + GC].reshape(2, 128).T.copy()
        bk2 = bk[c0:c0 + GC].reshape(2, 128).T.copy()
        in_maps.append({
                "xqT": xT[b],
                "xkvT": kvT[b],
                "wq": np.ascontiguousarray(Wq[:, c0:c0 + GC]).astype(BF16NP),
                "wk": np.ascontiguousarray(Wk[:, c0:c0 + GC]).astype(BF16NP),
                "wvp": wvp.astype(BF16NP),
                "wo": np.ascontiguousarray(Wo[c0:c0 + GC, :]).astype(BF16NP),
                "bq2": np.ascontiguousarray(bq2),
                "bk2": np.ascontiguousarray(bk2),
                "ones64": ones64,
        })
    return in_maps


def kernel(query_input, kv_input, Wq, bq, Wkv, bkv, Wo, bo):
    global _nc_cache
    from concourse import bass_utils

    if _nc_cache is None:
        _nc_cache = _build_nc()
    nc = _nc_cache

    Wkv = np.asarray(Wkv, dtype=np.float32)
    Wo = np.asarray(Wo, dtype=np.float32)
    bo = np.asarray(bo, dtype=np.float32)
    bv = np.asarray(bkv, np.float32)[D:]

    in_maps = build_in_maps(dict(
        query_input=query_input, kv_input=kv_input, Wq=Wq, bq=bq,
        Wkv=Wkv, bkv=bkv, Wo=Wo))

    res = bass_utils.run_bass_kernel_spmd(nc, in_maps,
                                          core_ids=list(range(N_CORES)))

    # gather: sum the 4 head-group partials per batch; add biases the device
    # left out (bo, and bv which passes through Wo since softmax rows sum to 1)
    tail = bv @ Wo + bo
    out = np.empty((B, SQ, D), np.float32)
    for b in range(B):
        acc = res.results[b * G + 0]["out_p"].astype(np.float32).copy()
        for g in range(1, G):
                acc += res.results[b * G + g]["out_p"]
        out[b] = acc + tail[None, :]
    return out



# revision 7
# speedup vs baseline: 1.1213x; 1.1213x over previous
"""Multi-head cross attention on 8 Trainium2 NeuronCores.

Sharding: core c = b*4 + g handles batch b (of 2) and head-group g (4 heads
of the 16).  Each core projects Q/K/V for its 4 heads, runs attention, and
computes a partial output projection with its 256 rows of Wo; the host sums
the 4 partials per batch (plus bo and the bv@Wo term, exact because softmax
rows sum to 1).

v2 schedule (vs baseline): the whole kernel is one software-pipelined
stream.  All input DMAs are issued up front (xkv first, then xq); KT
matmuls chase the xkv tiles as they land; attention for head-pair 0 starts
as soon as KT + V' + the first Q quarter are done.  The remaining QT
chunks and the output projection run as PE filler inside the ACT-bound
attention loop (exp gates the loop at ~2us per fused pair of kv tiles).
Scores for two kv tiles are fused into one [128,2048] bf16 PSUM tile so a
single ACT Exp covers both (halves the per-instruction ACT overhead).
Row-sum reciprocals use the fast custom-DVE op (~5x the iterative divide),
and the normalize multiply reads o_ps PSUM directly (no staging copies).
"""

import sys

sys.path.insert(0, "/opt/trn_rl_repo")

import ml_dtypes
import numpy as np

BF16NP = ml_dtypes.bfloat16

B, SQ, SKV, D, H = 2, 2048, 2048, 1024, 16
DH = D // H          # 64
N_CORES = 8
G = 4                # head groups
HPG = H // G         # heads per group = 4
GC = HPG * DH        # group width = 256

_nc_cache = None


def _build_nc():
    import concourse.mybir as mybir
    import concourse.tile as tile
    from concourse import bacc

    F32 = mybir.dt.float32
    F32R = mybir.dt.float32r
    BF16 = mybir.dt.bfloat16
    AF = mybir.ActivationFunctionType
    MUL = mybir.AluOpType.mult

    nc = bacc.Bacc("TRN2", target_bir_lowering=False, debug=False,
                   num_devices=N_CORES)

    xqT_d = nc.dram_tensor("xqT", [D, SQ], BF16, kind="ExternalInput").ap()
    xkvT_d = nc.dram_tensor("xkvT", [D, SKV], BF16, kind="ExternalInput").ap()
    wq_d = nc.dram_tensor("wq", [D, GC], BF16, kind="ExternalInput").ap()
    wk_d = nc.dram_tensor("wk", [D, GC], BF16, kind="ExternalInput").ap()
    # Wv' with a zero column after each head's 64 (slots for the ones column)
    wvp_d = nc.dram_tensor("wvp", [D, HPG * 65], BF16, kind="ExternalInput").ap()
    wo_d = nc.dram_tensor("wo", [GC, D], BF16, kind="ExternalInput").ap()
    bq_d = nc.dram_tensor("bq2", [128, 2], F32, kind="ExternalInput").ap()
    bk_d = nc.dram_tensor("bk2", [128, 2], F32, kind="ExternalInput").ap()
    out_d = nc.dram_tensor("out_p", [SQ, D], F32, kind="ExternalOutput").ap()

    ND = D // 128        # 8 d-tiles (contraction over D)
    NJ = SKV // 128      # 16 kv tiles
    VW = HPG * 65        # 260, V' row width
    scale = 1.0 / float(np.sqrt(DH))

    with tile.TileContext(nc) as tc:
        with (
            tc.tile_pool(name="persist", bufs=1) as pp,
            tc.tile_pool(name="work", bufs=1) as wk_pool,
        ):
            # ---- persistent tiles -------------------------------------
            qt_sb = pp.tile([128, 2 * SQ], BF16, tag="qt_sb")
            kt_sb = pp.tile([128, 2 * SKV], BF16, tag="kt_sb")
            vp_sb = pp.tile([128, NJ * VW + 63], BF16, tag="vp_sb")
            o_sbA = pp.tile([128, 2 * 1024], BF16, tag="o_sbA")
            o_sbB = pp.tile([128, 2 * 1024], BF16, tag="o_sbB")
            bq_sb = pp.tile([128, 2], F32, tag="bq_sb")
            bk_sb = pp.tile([128, 2], F32, tag="bk_sb")
            wq_sb = pp.tile([128, ND * GC], BF16, tag="wq_sb")
            wk_sb = pp.tile([128, ND * GC], BF16, tag="wk_sb")
            wvp_sb = pp.tile([128, ND * VW], BF16, tag="wvp_sb")
            wo_sb = pp.tile([128, 2 * D], BF16, tag="wo_sb")

            # ---- all DMAs issued up front -----------------------------
            nc.sync.dma_start(out=bq_sb[:], in_=bq_d[:])
            nc.sync.dma_start(out=bk_sb[:], in_=bk_d[:])
            for d in range(ND):
                nc.sync.dma_start(
                    out=wk_sb[:, d * GC:(d + 1) * GC],
                    in_=wk_d[d * 128:(d + 1) * 128, :])
            for d in range(ND):
                nc.sync.dma_start(
                    out=wvp_sb[:, d * VW:(d + 1) * VW],
                    in_=wvp_d[d * 128:(d + 1) * 128, :])
            for d in range(ND):
                nc.sync.dma_start(
                    out=wq_sb[:, d * GC:(d + 1) * GC],
                    in_=wq_d[d * 128:(d + 1) * 128, :])
            nc.sync.dma_start(
                out=wo_sb[:].rearrange("p (t n) -> p t n", t=2),
                in_=wo_d.rearrange("(t p) n -> p t n", p=128),
            )
            # big inputs on the gpsimd queue: xkv first (KT chases the
            # arriving d-tiles), then xq (needed a bit later for QT)
            xkv = []
            for d in range(ND):
                t = wk_pool.tile([128, SKV], BF16, tag=f"xkv{d}", name=f"xkv{d}")
                nc.gpsimd.dma_start(out=t[:], in_=xkvT_d[d * 128:(d + 1) * 128, :])
                xkv.append(t)
            xq_tiles = []
            for d in range(ND):
                t = wk_pool.tile([128, SQ], BF16, tag=f"xq{d}", name=f"xq{d}")
                nc.gpsimd.dma_start(out=t[:], in_=xqT_d[d * 128:(d + 1) * 128, :])
                xq_tiles.append(t)

            # ---- phase 1: KT (d-outer), V', QT(pair0, qq0) ------------
            with tc.tile_pool(name="psA", bufs=1, space="PSUM") as psA:
                pk = {}
                for p in range(2):
                    for qc in range(4):
                        pk[p, qc] = psA.tile([128, 512], F32, tag="pk",
                                             bufs=8, name=f"pk{p}{qc}")
                for d in range(ND):
                    for p in range(2):
                        for qc in range(4):
                            nc.tensor.matmul(
                                pk[p, qc][:],
                                wk_sb[:, d * GC + p * 128:d * GC + (p + 1) * 128],
                                xkv[d][:, qc * 512:(qc + 1) * 512],
                                start=(d == 0), stop=(d == ND - 1),
                            )
                for p in range(2):
                    for qc in range(4):
                        nc.scalar.activation(
                            kt_sb[:, p * SKV + qc * 512:p * SKV + (qc + 1) * 512],
                            pk[p, qc][:], AF.Identity, bias=bk_sb[:, p:p + 1])
                # V' (16 kv tiles, accumulate over d)
                for j in range(NJ):
                    pv = psA.tile([128, VW], F32, tag="pk", bufs=8,
                                  name=f"pv{j}")
                    for d in range(ND):
                        nc.tensor.matmul(
                            pv[:],
                            xkv[d][:, j * 128:(j + 1) * 128],
                            wvp_sb[:, d * VW:(d + 1) * VW],
                            start=(d == 0), stop=(d == ND - 1),
                        )
                    nc.vector.tensor_copy(vp_sb[:, j * VW:(j + 1) * VW], pv[:])
                    if j == NJ - 1:
                        # ones columns (stride-65 view hits col 64 per head)
                        oc = vp_sb[:, 64:NJ * VW:65]
                        nc.scalar.activation(oc, oc, AF.Copy, scale=0.0,
                                             bias=1.0)
                        # zero tail pad
                        nc.scalar.activation(vp_sb[:, NJ * VW:NJ * VW + 63],
                                             pv[:, 0:63], AF.Copy, scale=0.0)
                # QT chunk (pair 0, qq 0) so attention can start; its d-MMs
                # are interleaved above via the V' loop? No — emitted here,
                # but each d waits only on its own xq tile, so the stream
                # naturally chases the xq DMAs.
                pq00 = psA.tile([128, 512], F32, tag="pk", bufs=8, name="pq00")
                for d in range(ND):
                    nc.tensor.matmul(
                        pq00[:],
                        wq_sb[:, d * GC:d * GC + 128],
                        xq_tiles[d][:, 0:512],
                        start=(d == 0), stop=(d == ND - 1),
                    )
                nc.scalar.activation(qt_sb[:, 0:512], pq00[:],
                                     AF.Identity, bias=bq_sb[:, 0:1])

            # ---- attention + interleaved QT-rest + outproj ------------
            # PSUM budget: st 2x[128,2048]bf16 = 4 banks, o_ps 2x[128,512]f32
            # = 2 banks, px (QT chunks / outproj) 2x[128,512]f32 = 2 banks.
            with (
                tc.tile_pool(name="attn", bufs=1) as at,
                tc.tile_pool(name="psB", bufs=1, space="PSUM") as psB,
            ):
                # filler generators: emit_filler() emits up to n PE matmuls
                # from the pending queue (QT chunks, then outproj tiles)
                qt_chunks = [(p, qc) for p in range(2) for qc in range(4)
                             if not (p == 0 and qc == 0)]
                filler_state = {"cur": None, "d": 0, "queue": list(qt_chunks)}

                def emit_qt_filler(n):
                    st = filler_state
                    emitted = 0
                    while emitted < n:
                        if st["cur"] is None:
                            if not st["queue"]:
                                return emitted
                            st["cur"] = st["queue"].pop(0)
                            st["d"] = 0
                            st["ps"] = psB.tile([128, 512], F32, tag="px",
                                                bufs=2,
                                                name=f"pq{st['cur']}")
                        p, qc = st["cur"]
                        dd = st["d"]
                        nc.tensor.matmul(
                            st["ps"][:],
                            wq_sb[:, dd * GC + p * 128:dd * GC + (p + 1) * 128],
                            xq_tiles[dd][:, qc * 512:(qc + 1) * 512],
                            start=(dd == 0), stop=(dd == ND - 1),
                        )
                        emitted += 1
                        st["d"] += 1
                        if st["d"] == ND:
                            blk = slice(p * SQ + qc * 512,
                                        p * SQ + (qc + 1) * 512)
                            nc.scalar.activation(
                                qt_sb[:, blk], st["ps"][:],
                                AF.Identity, bias=bq_sb[:, p:p + 1])
                            st["cur"] = None
                    return emitted

                def emit_outproj_tile(s, n2):
                    po = psB.tile([128, 512], F32, tag="px", bufs=2,
                                  name=f"po{s}{n2}")
                    o_half = o_sbA if s < 8 else o_sbB
                    s8 = s % 8
                    for tt in range(2):
                        nc.tensor.matmul(
                            po[:],
                            o_half[:, tt * 1024 + s8 * 128:
                                   tt * 1024 + (s8 + 1) * 128],
                            wo_sb[:, tt * D + n2 * 512:
                                  tt * D + n2 * 512 + 512],
                            start=(tt == 0), stop=(tt == 1),
                        )
                    ob = at.tile([128, 512], F32, tag="ob",
                                 bufs=3, name=f"ob{s}{n2}")
                    nc.vector.tensor_copy(ob[:], po[:])
                    nc.sync.dma_start(
                        out=out_d[s * 128:(s + 1) * 128,
                                  n2 * 512:(n2 + 1) * 512],
                        in_=ob[:])

                # outproj work list refilled per qq once both pairs done
                op_queue = []

                def emit_op_filler(n):
                    emitted = 0
                    while emitted < n and op_queue:
                        s, n2 = op_queue.pop(0)
                        emit_outproj_tile(s, n2)
                        emitted += 2
                    return emitted

                for t in range(2):          # head pair
                    for qq in range(4):     # q quarter (512)
                        o_ps = {}
                        for hp in range(2):
                            o_ps[hp] = psB.tile(
                                [128, 512], F32, tag="o_ps", bufs=2,
                                name=f"o_ps{t}{qq}{hp}")
                        for j in range(NJ):
                            # score tile [kv 128, 2 heads x 512 q]; the two
                            # hp matmuls hit disjoint PE row groups and
                            # separate PSUM banks, so they run concurrently
                            st = psB.tile([128, 1024], F32, tag="st",
                                          bufs=2, name=f"st{t}{qq}{j}")
                            for hp in range(2):
                                nc.tensor.matmul(
                                    st[:, hp * 512:(hp + 1) * 512],
                                    kt_sb[hp * 64:(hp + 1) * 64,
                                          t * SKV + j * 128:
                                          t * SKV + (j + 1) * 128],
                                    qt_sb[hp * 64:(hp + 1) * 64,
                                          t * SQ + qq * 512:
                                          t * SQ + (qq + 1) * 512],
                                    start=True, stop=True,
                                )
                            # PE filler while ACT runs exp: QT chunks in
                            # t0, outproj of the previous qq in t1
                            if t == 0:
                                emit_qt_filler(1)
                            else:
                                if emit_op_filler(1) == 0:
                                    emit_qt_filler(1)
                            p_t = at.tile([128, 1024], BF16, tag="pt",
                                          bufs=6, name=f"pt{t}{qq}{j}")
                            nc.scalar.activation(p_t[:], st[:],
                                                 AF.Exp, scale=scale)
                            for hp in range(2):
                                h = 2 * t + hp
                                nc.tensor.matmul(
                                    o_ps[hp][:],
                                    vp_sb[:, j * VW + h * 65:
                                          j * VW + h * 65 + 128],
                                    p_t[:, hp * 512:(hp + 1) * 512],
                                    start=(j == 0), stop=(j == NJ - 1),
                                )
                        # normalize: stage rows 0-64 (O + row-sum) to SBUF in
                        # one copy so the PSUM accumulator frees fast, then
                        # fast-reciprocal the row-sums and scale
                        o_half = o_sbA if qq < 2 else o_sbB
                        col = t * 1024 + (qq % 2) * 512
                        ot = {}
                        rs = {}
                        for hp in range(2):
                            ot[hp] = at.tile([64, 512], F32, tag="ot",
                                             bufs=4, name=f"ot{t}{qq}{hp}")
                            nc.vector.tensor_copy(ot[hp][:], o_ps[hp][0:64, :])
                            rs[hp] = at.tile([1, 512], F32, tag="rs", bufs=4,
                                             name=f"rs{t}{qq}{hp}")
                            nc.vector.tensor_copy(rs[hp][:], o_ps[hp][64:65, :])
                        for hp in range(2):
                            rcp = at.tile([1, 512], F32, tag="rcp",
                                          bufs=4, name=f"rcp{t}{qq}{hp}")
                            nc.vector.reciprocal_approx_fast(
                                rcp[:], rs[hp][:])
                            bcs = at.tile([64, 512], F32, tag="bcs",
                                          bufs=4, name=f"bcs{t}{qq}{hp}")
                            nc.gpsimd.partition_broadcast(
                                bcs[:], rcp[:], channels=64)
                            nc.vector.tensor_tensor(
                                out=o_half[hp * 64:(hp + 1) * 64,
                                           col:col + 512],
                                in0=ot[hp][0:64, :], in1=bcs[:],
                                op=MUL)
                        if t == 1:
                            # queue outproj for this q quarter (both pairs
                            # now normalized); it gap-fills the next qq's
                            # attention, or runs as tail for qq==3
                            for s in range(qq * 4, qq * 4 + 4):
                                for n2 in range(2):
                                    op_queue.append((s, n2))
                # drain any remaining filler work
                while emit_op_filler(2) or emit_qt_filler(2):
                    pass

    nc.compile()
    return nc


def build_in_maps(inputs):
    query_input = np.asarray(inputs["query_input"], dtype=np.float32)
    kv_input = np.asarray(inputs["kv_input"], dtype=np.float32)
    Wq = np.asarray(inputs["Wq"], dtype=np.float32)
    bq = np.asarray(inputs["bq"], dtype=np.float32)
    Wkv = np.asarray(inputs["Wkv"], dtype=np.float32)
    bkv = np.asarray(inputs["bkv"], dtype=np.float32)
    Wo = np.asarray(inputs["Wo"], dtype=np.float32)

    Wk = Wkv[:, :D]
    Wv = Wkv[:, D:]
    bk = bkv[:D]

    xT = [np.ascontiguousarray(query_input[b].T).astype(BF16NP) for b in range(B)]
    kvT = [np.ascontiguousarray(kv_input[b].T).astype(BF16NP) for b in range(B)]

    in_maps = []
    for c in range(N_CORES):
        b, g = divmod(c, G)
        c0 = g * GC
        wvp = np.zeros((D, HPG * 65), np.float32)
        for h in range(HPG):
                wvp[:, h * 65:h * 65 + 64] = Wv[:, c0 + h * DH:c0 + (h + 1) * DH]
        bq2 = bq[c0:c0 + GC].reshape(2, 128).T.copy()
        bk2 = bk[c0:c0 + GC].reshape(2, 128).T.copy()
        in_maps.append({
                "xqT": xT[b],
                "xkvT": kvT[b],
                "wq": np.ascontiguousarray(Wq[:, c0:c0 + GC]).astype(BF16NP),
                "wk": np.ascontiguousarray(Wk[:, c0:c0 + GC]).astype(BF16NP),
                "wvp": wvp.astype(BF16NP),
                "wo": np.ascontiguousarray(Wo[c0:c0 + GC, :]).astype(BF16NP),
                "bq2": np.ascontiguousarray(bq2),
                "bk2": np.ascontiguousarray(bk2),
        })
    return in_maps


def kernel(query_input, kv_input, Wq, bq, Wkv, bkv, Wo, bo):
    global _nc_cache
    from concourse import bass_utils

    if _nc_cache is None:
        _nc_cache = _build_nc()
    nc = _nc_cache

    Wkv = np.asarray(Wkv, dtype=np.float32)
    Wo = np.asarray(Wo, dtype=np.float32)
    bo = np.asarray(bo, dtype=np.float32)
    bv = np.asarray(bkv, np.float32)[D:]

    in_maps = build_in_maps(dict(
        query_input=query_input, kv_input=kv_input, Wq=Wq, bq=bq,
        Wkv=Wkv, bkv=bkv, Wo=Wo))

    res = bass_utils.run_bass_kernel_spmd(nc, in_maps,
                                          core_ids=list(range(N_CORES)))

    # gather: sum the 4 head-group partials per batch; add biases the device
    # left out (bo, and bv which passes through Wo since softmax rows sum to 1)
    tail = bv @ Wo + bo
    out = np.empty((B, SQ, D), np.float32)
    for b in range(B):
        acc = res.results[b * G + 0]["out_p"].astype(np.float32).copy()
        for g in range(1, G):
                acc += res.results[b * G + g]["out_p"]
        out[b] = acc + tail[None, :]
    return out


# revision 12
# speedup vs baseline: 1.1776x; 1.0502x over previous
"""Multi-head cross attention on 8 Trainium2 NeuronCores.

Sharding: core c = b*4 + g handles batch b (of 2) and head-group g (4 heads
of the 16).  Each core projects Q/K/V for its 4 heads, runs attention, and
computes partial output projections with its 256 rows of Wo split into two
128-row head-pair chunks; the host sums the 8 bf16 partials per batch (plus
bo and the bv@Wo term, exact because softmax rows sum to 1).

v3 schedule: one software-pipelined stream.  All input DMAs are issued up
front (weights, then xkv, then xq); KT matmuls chase the xkv tiles as they
land; V' runs for kv tiles 0-8 and the first two Q chunks, then attention
starts (~45us in).  Everything else - the remaining V' tiles, the remaining
QT chunks, and all output-projection tiles - runs as PE filler inside the
ACT-bound attention loop (exp gates it at ~1.2us per kv tile).  AV matmuls
lag the score matmuls by two kv tiles so the PSUM-accumulator handoff at
block boundaries never stalls the PE queue ahead of the exp stream.  The
output projection is split by head pair (one K=128 matmul per tile) so each
half runs as soon as its pair is normalized; only the last quarter's second
half remains as tail.  Row-sum reciprocals use the fast custom-DVE op.
"""

import sys

sys.path.insert(0, "/opt/trn_rl_repo")

import ml_dtypes
import numpy as np

BF16NP = ml_dtypes.bfloat16

B, SQ, SKV, D, H = 2, 2048, 2048, 1024, 16
DH = D // H          # 64
N_CORES = 8
G = 4                # head groups
HPG = H // G         # heads per group = 4
GC = HPG * DH        # group width = 256

_nc_cache = None


def _build_nc():
    import concourse.mybir as mybir
    import concourse.tile as tile
    from concourse import bacc

    F32 = mybir.dt.float32
    BF16 = mybir.dt.bfloat16
    AF = mybir.ActivationFunctionType
    MUL = mybir.AluOpType.mult

    nc = bacc.Bacc("TRN2", target_bir_lowering=False, debug=False,
                   num_devices=N_CORES)

    xqT_d = nc.dram_tensor("xqT", [D, SQ], BF16, kind="ExternalInput").ap()
    xkvT_d = nc.dram_tensor("xkvT", [D, SKV], BF16, kind="ExternalInput").ap()
    wq_d = nc.dram_tensor("wq", [D, GC], BF16, kind="ExternalInput").ap()
    wk_d = nc.dram_tensor("wk", [D, GC], BF16, kind="ExternalInput").ap()
    # Wv' with a zero column after each head's 64 (slots for the ones column)
    wvp_d = nc.dram_tensor("wvp", [D, HPG * 65], BF16, kind="ExternalInput").ap()
    wo_d = nc.dram_tensor("wo", [GC, D], BF16, kind="ExternalInput").ap()
    bq_d = nc.dram_tensor("bq2", [128, 2], F32, kind="ExternalInput").ap()
    bk_d = nc.dram_tensor("bk2", [128, 2], F32, kind="ExternalInput").ap()
    out0_d = nc.dram_tensor("out_p0", [SQ, D], BF16, kind="ExternalOutput").ap()
    out1_d = nc.dram_tensor("out_p1", [SQ, D], BF16, kind="ExternalOutput").ap()
    out_ds = [out0_d, out1_d]

    ND = D // 128        # 8 d-tiles (contraction over D)
    NJ = SKV // 128      # 16 kv tiles
    VW = HPG * 65        # 260, V' row width
    NJ_PRE = 9           # V' tiles computed before attention starts
    scale = 1.0 / float(np.sqrt(DH))

    with tile.TileContext(nc) as tc:
        with (
            tc.tile_pool(name="persist", bufs=1) as pp,
            tc.tile_pool(name="work", bufs=1) as wk_pool,
        ):
            # ---- persistent tiles -------------------------------------
            qt_sb = pp.tile([128, 2 * SQ], BF16, tag="qt_sb")
            kt_sb = pp.tile([128, 2 * SKV], BF16, tag="kt_sb")
            vp_sb = pp.tile([128, NJ * VW + 63], BF16, tag="vp_sb")
            o_sbA = pp.tile([128, 2 * 1024], BF16, tag="o_sbA")
            o_sbB = pp.tile([128, 2 * 1024], BF16, tag="o_sbB")
            bq_sb = pp.tile([128, 2], F32, tag="bq_sb")
            bk_sb = pp.tile([128, 2], F32, tag="bk_sb")
            wq_sb = pp.tile([128, ND * GC], BF16, tag="wq_sb")
            wk_sb = pp.tile([128, ND * GC], BF16, tag="wk_sb")
            wvp_sb = pp.tile([128, ND * VW], BF16, tag="wvp_sb")
            wo_sb = pp.tile([128, 2 * D], BF16, tag="wo_sb")
            dum = pp.tile([128, 512], BF16, tag="dum")
            dscr = pp.tile([128, 16], F32, tag="dscr")

            # ---- all DMAs issued up front -----------------------------
            nc.sync.dma_start(out=bq_sb[:], in_=bq_d[:])
            nc.sync.dma_start(out=bk_sb[:], in_=bk_d[:])
            for d in range(ND):
                nc.sync.dma_start(
                    out=wk_sb[:, d * GC:(d + 1) * GC],
                    in_=wk_d[d * 128:(d + 1) * 128, :])
            for d in range(ND):
                nc.sync.dma_start(
                    out=wvp_sb[:, d * VW:(d + 1) * VW],
                    in_=wvp_d[d * 128:(d + 1) * 128, :])
            for d in range(ND):
                nc.sync.dma_start(
                    out=wq_sb[:, d * GC:(d + 1) * GC],
                    in_=wq_d[d * 128:(d + 1) * 128, :])
            nc.sync.dma_start(
                out=wo_sb[:].rearrange("p (t n) -> p t n", t=2),
                in_=wo_d.rearrange("(t p) n -> p t n", p=128),
            )
            xkv = []
            for d in range(ND):
                t = wk_pool.tile([128, SKV], BF16, tag=f"xkv{d}", name=f"xkv{d}")
                nc.gpsimd.dma_start(out=t[:], in_=xkvT_d[d * 128:(d + 1) * 128, :])
                xkv.append(t)
            xq_tiles = []
            for d in range(ND):
                t = wk_pool.tile([128, SQ], BF16, tag=f"xq{d}", name=f"xq{d}")
                nc.gpsimd.dma_start(out=t[:], in_=xqT_d[d * 128:(d + 1) * 128, :])
                xq_tiles.append(t)

            # ---- warmup: keep the PE busy through the DMA head so the
            # HAM clock gate opens, and pull the ACT exp table load off
            # the critical path
            nc.vector.memset(dum[:], 1.0)
            with tc.tile_pool(name="psW", bufs=1, space="PSUM") as psW:
                wps = psW.tile([128, 512], F32, tag="wps")
                for i in range(10):
                    nc.tensor.matmul(wps[:], dum[:, 0:128], dum[:],
                                     start=True, stop=True)
                nc.scalar.activation(dscr[:], dum[:, 0:16], AF.Exp, scale=1.0)

            # ---- phase 1: KT (d-outer), V' j0-8, QT chunks (0,0),(0,1)
            def qt_drain(ps, p, qc):
                blk = slice(p * SQ + qc * 512, p * SQ + (qc + 1) * 512)
                nc.vector.tensor_scalar_add(
                    qt_sb[:, blk], ps[:], bq_sb[:, p:p + 1])

            vp_done = {}          # j -> pv tile (for filler bookkeeping)

            def vp_copy(j, pv):
                nc.vector.tensor_copy(vp_sb[:, j * VW:(j + 1) * VW], pv[:])
                nc.gpsimd.memset(vp_sb[:, j * VW + 64:(j + 1) * VW:65], 1.0)

            with tc.tile_pool(name="psA", bufs=1, space="PSUM") as psA:
                pk = {}
                for p in range(2):
                    for qc in range(4):
                        pk[p, qc] = psA.tile([128, 512], F32, tag="pk",
                                             bufs=8, name=f"pk{p}{qc}")
                for d in range(ND):
                    for p in range(2):
                        for qc in range(4):
                            nc.tensor.matmul(
                                pk[p, qc][:],
                                wk_sb[:, d * GC + p * 128:d * GC + (p + 1) * 128],
                                xkv[d][:, qc * 512:(qc + 1) * 512],
                                start=(d == 0), stop=(d == ND - 1),
                            )
                for p in range(2):
                    for qc in range(4):
                        nc.scalar.activation(
                            kt_sb[:, p * SKV + qc * 512:p * SKV + (qc + 1) * 512],
                            pk[p, qc][:], AF.Identity, bias=bk_sb[:, p:p + 1])
                # V' j0-8
                for j in range(NJ_PRE):
                    pv = psA.tile([128, VW], F32, tag="pk", bufs=8,
                                  name=f"pv{j}")
                    for d in range(ND):
                        nc.tensor.matmul(
                            pv[:],
                            xkv[d][:, j * 128:(j + 1) * 128],
                            wvp_sb[:, d * VW:(d + 1) * VW],
                            start=(d == 0), stop=(d == ND - 1),
                        )
                    vp_copy(j, pv)
                nc.gpsimd.memset(vp_sb[:, NJ * VW:NJ * VW + 63], 0.0)
                # QT chunks (0,0) and (0,1): qq0 and qq1 of pair 0
                for qc in range(2):
                    pq = psA.tile([128, 512], F32, tag="pk", bufs=8,
                                  name=f"pq0{qc}")
                    for d in range(ND):
                        nc.tensor.matmul(
                            pq[:],
                            wq_sb[:, d * GC:d * GC + 128],
                            xq_tiles[d][:, qc * 512:(qc + 1) * 512],
                            start=(d == 0), stop=(d == ND - 1),
                        )
                    qt_drain(pq, 0, qc)

            # ---- attention with static filler schedule ----------------
            # PSUM: st 2x[128,1024]f32 = 4 banks, o_ps 2x[128,512] = 2,
            # px (V' tail / QT chunks / outproj) 2x[128,512] = 2.
            with (
                tc.tile_pool(name="attn", bufs=1) as at,
                tc.tile_pool(name="psB", bufs=1, space="PSUM") as psB,
            ):
                # --- filler emitters ---------------------------------
                def emit_vp_half(j, half):
                    if half == 0:
                        vp_done[j] = psB.tile([128, VW], F32,
                                              tag=("pxq" if j % 2 else "pxo"),
                                              bufs=1, name=f"pvf{j}")
                    pv = vp_done[j]
                    for d in range(4 * half, 4 * half + 4):
                        nc.tensor.matmul(
                            pv[:],
                            xkv[d][:, j * 128:(j + 1) * 128],
                            wvp_sb[:, d * VW:(d + 1) * VW],
                            start=(d == 0), stop=(d == ND - 1),
                        )
                    if half == 1:
                        vp_copy(j, pv)

                qt_state = {}

                def emit_qt_d(p, qc, d):
                    if d == 0:
                        qt_state[p, qc] = psB.tile([128, 512], F32, tag="pxq",
                                                   bufs=1, name=f"pq{p}{qc}")
                    ps = qt_state[p, qc]
                    nc.tensor.matmul(
                        ps[:],
                        wq_sb[:, d * GC + p * 128:d * GC + (p + 1) * 128],
                        xq_tiles[d][:, qc * 512:(qc + 1) * 512],
                        start=(d == 0), stop=(d == ND - 1),
                    )
                    if d == ND - 1:
                        qt_drain(ps, p, qc)

                def emit_outproj(s, n2, tt):
                    po = psB.tile([128, 512], F32, tag="pxo", bufs=1,
                                  name=f"po{s}{n2}{tt}")
                    o_half = o_sbA if s < 8 else o_sbB
                    s8 = s % 8
                    nc.tensor.matmul(
                        po[:],
                        o_half[:, tt * 1024 + s8 * 128:
                               tt * 1024 + (s8 + 1) * 128],
                        wo_sb[:, tt * D + n2 * 512:tt * D + n2 * 512 + 512],
                        start=True, stop=True,
                    )
                    ob = at.tile([128, 512], BF16, tag="ob",
                                 bufs=3, name=f"ob{s}{n2}{tt}")
                    nc.vector.tensor_copy(ob[:], po[:])
                    nc.sync.dma_start(
                        out=out_ds[tt][s * 128:(s + 1) * 128,
                                       n2 * 512:(n2 + 1) * 512],
                        in_=ob[:])

                def opj_items(qq, tt):
                    return [("op", s, n2, tt)
                            for s in range(qq * 4, qq * 4 + 4)
                            for n2 in range(2)]

                def qt_items(p, qc):
                    return [("qt", p, qc, d) for d in range(ND)]

                # per-window filler lists; windows are (t, qq), 16 slots
                fillers = {
                    (0, 0): [("vp", j, h) for j in range(NJ_PRE, NJ)
                             for h in range(2)],
                    (0, 1): qt_items(0, 2) + opj_items(0, 0),
                    (0, 2): qt_items(0, 3) + opj_items(1, 0),
                    (0, 3): qt_items(1, 0) + opj_items(2, 0),
                    (1, 0): qt_items(1, 1) + opj_items(3, 0),
                    (1, 1): qt_items(1, 2) + opj_items(0, 1),
                    (1, 2): qt_items(1, 3) + opj_items(1, 1),
                    (1, 3): opj_items(2, 1),
                }

                def emit_filler_item(it):
                    kind = it[0]
                    if kind == "vp":
                        emit_vp_half(it[1], it[2])
                    elif kind == "qt":
                        emit_qt_d(it[1], it[2], it[3])
                    else:
                        emit_outproj(it[1], it[2], it[3])

                def emit_av(t, o_ps, p_ts, j):
                    for hp in range(2):
                        h = 2 * t + hp
                        nc.tensor.matmul(
                            o_ps[hp][:],
                            vp_sb[:, j * VW + h * 65:j * VW + h * 65 + 128],
                            p_ts[j][:, hp * 512:(hp + 1) * 512],
                            start=(j == 0), stop=(j == NJ - 1),
                        )

                for t in range(2):          # head pair
                    for qq in range(4):     # q quarter (512)
                        win = list(fillers[(t, qq)])
                        # interleave QT d-matmuls with outproj tiles so the
                        # px pool's two buffers alternate cleanly
                        if len(win) == 16 and win[0][0] == "qt":
                            win = [win[i // 2 + (len(win) // 2) * (i % 2)]
                                   for i in range(16)]
                        o_ps = {}
                        for hp in range(2):
                            o_ps[hp] = psB.tile(
                                [128, 512], F32, tag="o_ps", bufs=2,
                                name=f"o_ps{t}{qq}{hp}")
                        p_ts = {}
                        for j in range(NJ):
                            st = psB.tile([128, 1024], F32, tag="st",
                                          bufs=2, name=f"st{t}{qq}{j}")
                            for hp in range(2):
                                nc.tensor.matmul(
                                    st[:, hp * 512:(hp + 1) * 512],
                                    kt_sb[hp * 64:(hp + 1) * 64,
                                          t * SKV + j * 128:
                                          t * SKV + (j + 1) * 128],
                                    qt_sb[hp * 64:(hp + 1) * 64,
                                          t * SQ + qq * 512:
                                          t * SQ + (qq + 1) * 512],
                                    start=True, stop=True,
                                )
                            if win:
                                emit_filler_item(win.pop(0))
                            p_ts[j] = at.tile([128, 1024], BF16, tag="pt",
                                              bufs=6, name=f"pt{t}{qq}{j}")
                            nc.scalar.activation(p_ts[j][:], st[:],
                                                 AF.Exp, scale=scale)
                            # AV lags by two kv tiles: the PSUM accumulator
                            # handoff at block starts stays off the PE
                            # critical path
                            if j >= 2:
                                emit_av(t, o_ps, p_ts, j - 2)
                        for j in (NJ - 2, NJ - 1):
                            emit_av(t, o_ps, p_ts, j)
                        while win:
                            emit_filler_item(win.pop(0))
                        # normalize
                        o_half = o_sbA if qq < 2 else o_sbB
                        col = t * 1024 + (qq % 2) * 512
                        ot, rs = {}, {}
                        for hp in range(2):
                            ot[hp] = at.tile([64, 512], F32, tag="ot",
                                             bufs=4, name=f"ot{t}{qq}{hp}")
                            nc.vector.tensor_copy(ot[hp][:], o_ps[hp][0:64, :])
                            rs[hp] = at.tile([1, 512], F32, tag="rs", bufs=4,
                                             name=f"rs{t}{qq}{hp}")
                            nc.vector.tensor_copy(rs[hp][:], o_ps[hp][64:65, :])
                        for hp in range(2):
                            rcp = at.tile([1, 512], F32, tag="rcp",
                                          bufs=4, name=f"rcp{t}{qq}{hp}")
                            nc.vector.reciprocal_approx_fast(rcp[:], rs[hp][:])
                            bcs = at.tile([64, 512], F32, tag="bcs",
                                          bufs=4, name=f"bcs{t}{qq}{hp}")
                            nc.gpsimd.partition_broadcast(
                                bcs[:], rcp[:], channels=64)
                            nc.vector.tensor_tensor(
                                out=o_half[hp * 64:(hp + 1) * 64,
                                           col:col + 512],
                                in0=ot[hp][:], in1=bcs[:],
                                op=MUL)
                # tail: last quarter's second-half outproj
                for it in opj_items(3, 1):
                    emit_filler_item(it)

    nc.compile()
    return nc


def build_in_maps(inputs):
    query_input = np.asarray(inputs["query_input"], dtype=np.float32)
    kv_input = np.asarray(inputs["kv_input"], dtype=np.float32)
    Wq = np.asarray(inputs["Wq"], dtype=np.float32)
    bq = np.asarray(inputs["bq"], dtype=np.float32)
    Wkv = np.asarray(inputs["Wkv"], dtype=np.float32)
    bkv = np.asarray(inputs["bkv"], dtype=np.float32)
    Wo = np.asarray(inputs["Wo"], dtype=np.float32)

    Wk = Wkv[:, :D]
    Wv = Wkv[:, D:]
    bk = bkv[:D]

    xT = [np.ascontiguousarray(query_input[b].T).astype(BF16NP) for b in range(B)]
    kvT = [np.ascontiguousarray(kv_input[b].T).astype(BF16NP) for b in range(B)]

    in_maps = []
    for c in range(N_CORES):
        b, g = divmod(c, G)
        c0 = g * GC
        wvp = np.zeros((D, HPG * 65), np.float32)
        for h in range(HPG):
                wvp[:, h * 65:h * 65 + 64] = Wv[:, c0 + h * DH:c0 + (h + 1) * DH]
        bq2 = bq[c0:c0 + GC].reshape(2, 128).T.copy()
        bk2 = bk[c0:c0 + GC].reshape(2, 128).T.copy()
        in_maps.append({
                "xqT": xT[b],
                "xkvT": kvT[b],
                "wq": np.ascontiguousarray(Wq[:, c0:c0 + GC]).astype(BF16NP),
                "wk": np.ascontiguousarray(Wk[:, c0:c0 + GC]).astype(BF16NP),
                "wvp": wvp.astype(BF16NP),
                "wo": np.ascontiguousarray(Wo[c0:c0 + GC, :]).astype(BF16NP),
                "bq2": np.ascontiguousarray(bq2),
                "bk2": np.ascontiguousarray(bk2),
        })
    return in_maps


def kernel(query_input, kv_input, Wq, bq, Wkv, bkv, Wo, bo):
    global _nc_cache
    from concourse import bass_utils

    if _nc_cache is None:
        _nc_cache = _build_nc()
    nc = _nc_cache

    Wkv = np.asarray(Wkv, dtype=np.float32)
    Wo = np.asarray(Wo, dtype=np.float32)
    bo = np.asarray(bo, dtype=np.float32)
    bv = np.asarray(bkv, np.float32)[D:]

    in_maps = build_in_maps(dict(
        query_input=query_input, kv_input=kv_input, Wq=Wq, bq=bq,
        Wkv=Wkv, bkv=bkv, Wo=Wo))

    res = bass_utils.run_bass_kernel_spmd(nc, in_maps,
                                          core_ids=list(range(N_CORES)))

    # gather: sum the 8 head-pair partials per batch; add biases the device
    # left out (bo, and bv which passes through Wo since softmax rows sum to 1)
    tail = bv @ Wo + bo
    out = np.empty((B, SQ, D), np.float32)
    for b in range(B):
        acc = res.results[b * G + 0]["out_p0"].astype(np.float32).copy()
        acc += res.results[b * G + 0]["out_p1"]
        for g in range(1, G):
                acc += res.results[b * G + g]["out_p0"]
                acc += res.results[b * G + g]["out_p1"]
        out[b] = acc + tail[None, :]
    return out


# revision 13
# speedup vs baseline: 1.2089x; 1.0266x over previous
"""Multi-head cross attention on 8 Trainium2 NeuronCores.

Sharding: core c = b*4 + g handles batch b (of 2) and head-group g (4 heads
of the 16).  Each core projects Q/K/V for its 4 heads, runs attention, and
computes partial output projections with its 256 rows of Wo split into two
128-row head-pair chunks; the host sums the 8 bf16 partials per batch (plus
bo and the bv@Wo term, exact because softmax rows sum to 1).

v3 schedule: one software-pipelined stream.  All input DMAs are issued up
front (weights, then xkv, then xq); KT matmuls chase the xkv tiles as they
land; V' runs for kv tiles 0-8 and the first two Q chunks, then attention
starts (~45us in).  Everything else - the remaining V' tiles, the remaining
QT chunks, and all output-projection tiles - runs as PE filler inside the
ACT-bound attention loop (exp gates it at ~1.2us per kv tile).  AV matmuls
lag the score matmuls by two kv tiles so the PSUM-accumulator handoff at
block boundaries never stalls the PE queue ahead of the exp stream.  The
output projection is split by head pair (one K=128 matmul per tile) so each
half runs as soon as its pair is normalized; only the last quarter's second
half remains as tail.  Row-sum reciprocals use the fast custom-DVE op.
"""

import sys

sys.path.insert(0, "/opt/trn_rl_repo")

import ml_dtypes
import numpy as np

BF16NP = ml_dtypes.bfloat16

B, SQ, SKV, D, H = 2, 2048, 2048, 1024, 16
DH = D // H          # 64
N_CORES = 8
G = 4                # head groups
HPG = H // G         # heads per group = 4
GC = HPG * DH        # group width = 256

_nc_cache = None


def _build_nc():
    import concourse.mybir as mybir
    import concourse.tile as tile
    from concourse import bacc

    F32 = mybir.dt.float32
    BF16 = mybir.dt.bfloat16
    AF = mybir.ActivationFunctionType
    MUL = mybir.AluOpType.mult

    nc = bacc.Bacc("TRN2", target_bir_lowering=False, debug=False,
                   num_devices=N_CORES)

    xqT_d = nc.dram_tensor("xqT", [D, SQ], BF16, kind="ExternalInput").ap()
    xkvT_d = nc.dram_tensor("xkvT", [D, SKV], BF16, kind="ExternalInput").ap()
    wq_d = nc.dram_tensor("wq", [D, GC], BF16, kind="ExternalInput").ap()
    wk_d = nc.dram_tensor("wk", [D, GC], BF16, kind="ExternalInput").ap()
    # Wv' with a zero column after each head's 64 (slots for the ones column)
    wvp_d = nc.dram_tensor("wvp", [D, HPG * 65], BF16, kind="ExternalInput").ap()
    wo_d = nc.dram_tensor("wo", [GC, D], BF16, kind="ExternalInput").ap()
    bq_d = nc.dram_tensor("bq2", [128, 2], F32, kind="ExternalInput").ap()
    bk_d = nc.dram_tensor("bk2", [128, 2], F32, kind="ExternalInput").ap()
    out0_d = nc.dram_tensor("out_p0", [SQ, D], BF16, kind="ExternalOutput").ap()
    out1_d = nc.dram_tensor("out_p1", [SQ, D], BF16, kind="ExternalOutput").ap()
    out_ds = [out0_d, out1_d]

    ND = D // 128        # 8 d-tiles (contraction over D)
    NJ = SKV // 128      # 16 kv tiles
    VW = HPG * 65        # 260, V' row width
    NJ_PRE = 8           # V' tiles computed before attention starts
    scale = 1.0 / float(np.sqrt(DH))

    with tile.TileContext(nc) as tc:
        with (
            tc.tile_pool(name="persist", bufs=1) as pp,
            tc.tile_pool(name="work", bufs=1) as wk_pool,
        ):
            # ---- persistent tiles -------------------------------------
            qt_sb = pp.tile([128, 2 * SQ], BF16, tag="qt_sb")
            kt_sb = pp.tile([128, 2 * SKV], BF16, tag="kt_sb")
            vp_sb = pp.tile([128, NJ * VW + 63], BF16, tag="vp_sb")
            o_sbA = pp.tile([128, 2 * 1024], BF16, tag="o_sbA")
            o_sbB = pp.tile([128, 2 * 1024], BF16, tag="o_sbB")
            bq_sb = pp.tile([128, 2], F32, tag="bq_sb")
            bk_sb = pp.tile([128, 2], F32, tag="bk_sb")
            wq_sb = pp.tile([128, ND * GC], BF16, tag="wq_sb")
            wk_sb = pp.tile([128, ND * GC], BF16, tag="wk_sb")
            wvp_sb = pp.tile([128, ND * VW], BF16, tag="wvp_sb")
            wo_sb = pp.tile([128, 2 * D], BF16, tag="wo_sb")
            dum = pp.tile([128, 512], BF16, tag="dum")
            dscr = pp.tile([128, 16], F32, tag="dscr")

            # ---- all DMAs issued up front -----------------------------
            nc.sync.dma_start(out=bq_sb[:], in_=bq_d[:])
            nc.sync.dma_start(out=bk_sb[:], in_=bk_d[:])
            for d in range(ND):
                nc.sync.dma_start(
                    out=wk_sb[:, d * GC:(d + 1) * GC],
                    in_=wk_d[d * 128:(d + 1) * 128, :])
            for d in range(ND):
                nc.sync.dma_start(
                    out=wvp_sb[:, d * VW:(d + 1) * VW],
                    in_=wvp_d[d * 128:(d + 1) * 128, :])
            for d in range(ND):
                nc.sync.dma_start(
                    out=wq_sb[:, d * GC:(d + 1) * GC],
                    in_=wq_d[d * 128:(d + 1) * 128, :])
            nc.sync.dma_start(
                out=wo_sb[:].rearrange("p (t n) -> p t n", t=2),
                in_=wo_d.rearrange("(t p) n -> p t n", p=128),
            )
            xkv = []
            for d in range(ND):
                t = wk_pool.tile([128, SKV], BF16, tag=f"xkv{d}", name=f"xkv{d}")
                nc.gpsimd.dma_start(out=t[:], in_=xkvT_d[d * 128:(d + 1) * 128, :])
                xkv.append(t)
            # gate the xq loads behind xkv so the KT-feeding tiles get the
            # full HBM read bandwidth first (the DMA engines otherwise
            # interleave packets of every queued descriptor)
            dgate = pp.tile([1, 8], BF16, tag="dgate")
            nc.gpsimd.partition_broadcast(dgate[:], xkv[6][0:1, 0:8],
                                          channels=1)
            xq_tiles = []
            for d in range(ND):
                t = wk_pool.tile([128, SQ], BF16, tag=f"xq{d}", name=f"xq{d}")
                nc.gpsimd.dma_start(out=t[:], in_=xqT_d[d * 128:(d + 1) * 128, :])
                xq_tiles.append(t)

            # ---- warmup: keep the PE busy through the DMA head so the
            # HAM clock gate opens, and pull the ACT exp table load off
            # the critical path
            nc.vector.memset(dum[:], 1.0)
            with tc.tile_pool(name="psW", bufs=1, space="PSUM") as psW:
                wps = psW.tile([128, 512], F32, tag="wps")
                for i in range(10):
                    nc.tensor.matmul(wps[:], dum[:, 0:128], dum[:],
                                     start=True, stop=True)
                nc.scalar.activation(dscr[:], dum[:, 0:16], AF.Exp, scale=1.0)

            # ---- phase 1: KT (d-outer), V' j0-8, QT chunks (0,0),(0,1)
            def qt_drain(ps, p, qc):
                blk = slice(p * SQ + qc * 512, p * SQ + (qc + 1) * 512)
                nc.vector.tensor_scalar_add(
                    qt_sb[:, blk], ps[:], bq_sb[:, p:p + 1])

            vp_done = {}          # j -> pv tile (for filler bookkeeping)

            def vp_copy(j, pv):
                nc.vector.tensor_copy(vp_sb[:, j * VW:(j + 1) * VW], pv[:])
                nc.gpsimd.memset(vp_sb[:, j * VW + 64:(j + 1) * VW:65], 1.0)

            with tc.tile_pool(name="psA", bufs=1, space="PSUM") as psA:
                pk = {}
                for p in range(2):
                    for qc in range(4):
                        pk[p, qc] = psA.tile([128, 512], F32, tag="pk",
                                             bufs=8, name=f"pk{p}{qc}")
                for d in range(ND):
                    for p in range(2):
                        for qc in range(4):
                            nc.tensor.matmul(
                                pk[p, qc][:],
                                wk_sb[:, d * GC + p * 128:d * GC + (p + 1) * 128],
                                xkv[d][:, qc * 512:(qc + 1) * 512],
                                start=(d == 0), stop=(d == ND - 1),
                            )
                for p in range(2):
                    for qc in range(4):
                        nc.scalar.activation(
                            kt_sb[:, p * SKV + qc * 512:p * SKV + (qc + 1) * 512],
                            pk[p, qc][:], AF.Identity, bias=bk_sb[:, p:p + 1])
                # V' j0-8
                for j in range(NJ_PRE):
                    pv = psA.tile([128, VW], F32, tag="pk", bufs=8,
                                  name=f"pv{j}")
                    for d in range(ND):
                        nc.tensor.matmul(
                            pv[:],
                            xkv[d][:, j * 128:(j + 1) * 128],
                            wvp_sb[:, d * VW:(d + 1) * VW],
                            start=(d == 0), stop=(d == ND - 1),
                        )
                    vp_copy(j, pv)
                nc.gpsimd.memset(vp_sb[:, NJ * VW:NJ * VW + 63], 0.0)
                # QT chunks (0,0) and (0,1): qq0 and qq1 of pair 0
                for qc in range(2):
                    pq = psA.tile([128, 512], F32, tag="pk", bufs=8,
                                  name=f"pq0{qc}")
                    for d in range(ND):
                        nc.tensor.matmul(
                            pq[:],
                            wq_sb[:, d * GC:d * GC + 128],
                            xq_tiles[d][:, qc * 512:(qc + 1) * 512],
                            start=(d == 0), stop=(d == ND - 1),
                        )
                    qt_drain(pq, 0, qc)

            # ---- attention with static filler schedule ----------------
            # PSUM: st 2x[128,1024]f32 = 4 banks, o_ps 2x[128,512] = 2,
            # px (V' tail / QT chunks / outproj) 2x[128,512] = 2.
            with (
                tc.tile_pool(name="attn", bufs=1) as at,
                tc.tile_pool(name="psB", bufs=1, space="PSUM") as psB,
            ):
                # --- filler emitters ---------------------------------
                def emit_vp_half(j, half):
                    if half == 0:
                        vp_done[j] = psB.tile([128, VW], F32,
                                              tag=("pxq" if j % 2 else "pxo"),
                                              bufs=1, name=f"pvf{j}")
                    pv = vp_done[j]
                    for d in range(4 * half, 4 * half + 4):
                        nc.tensor.matmul(
                            pv[:],
                            xkv[d][:, j * 128:(j + 1) * 128],
                            wvp_sb[:, d * VW:(d + 1) * VW],
                            start=(d == 0), stop=(d == ND - 1),
                        )
                    if half == 1:
                        vp_copy(j, pv)

                qt_state = {}

                def emit_qt_d(p, qc, d):
                    if d == 0:
                        qt_state[p, qc] = psB.tile([128, 512], F32, tag="pxq",
                                                   bufs=1, name=f"pq{p}{qc}")
                    ps = qt_state[p, qc]
                    nc.tensor.matmul(
                        ps[:],
                        wq_sb[:, d * GC + p * 128:d * GC + (p + 1) * 128],
                        xq_tiles[d][:, qc * 512:(qc + 1) * 512],
                        start=(d == 0), stop=(d == ND - 1),
                    )
                    if d == ND - 1:
                        qt_drain(ps, p, qc)

                def emit_outproj(s, n2, tt):
                    po = psB.tile([128, 512], F32, tag="pxo", bufs=1,
                                  name=f"po{s}{n2}{tt}")
                    o_half = o_sbA if s < 8 else o_sbB
                    s8 = s % 8
                    nc.tensor.matmul(
                        po[:],
                        o_half[:, tt * 1024 + s8 * 128:
                               tt * 1024 + (s8 + 1) * 128],
                        wo_sb[:, tt * D + n2 * 512:tt * D + n2 * 512 + 512],
                        start=True, stop=True,
                    )
                    ob = at.tile([128, 512], BF16, tag="ob",
                                 bufs=3, name=f"ob{s}{n2}{tt}")
                    nc.vector.tensor_copy(ob[:], po[:])
                    nc.sync.dma_start(
                        out=out_ds[tt][s * 128:(s + 1) * 128,
                                       n2 * 512:(n2 + 1) * 512],
                        in_=ob[:])

                def opj_items(qq, tt):
                    return [("op", s, n2, tt)
                            for s in range(qq * 4, qq * 4 + 4)
                            for n2 in range(2)]

                def qt_items(p, qc):
                    return [("qt", p, qc, d) for d in range(ND)]

                # per-window filler lists; windows are (t, qq), 16 slots
                fillers = {
                    (0, 0): [("vp", j, h) for j in range(NJ_PRE, NJ)
                             for h in range(2)],
                    (0, 1): qt_items(0, 2) + opj_items(0, 0),
                    (0, 2): qt_items(0, 3) + opj_items(1, 0),
                    (0, 3): qt_items(1, 0) + opj_items(2, 0),
                    (1, 0): qt_items(1, 1) + opj_items(3, 0),
                    (1, 1): qt_items(1, 2) + opj_items(0, 1),
                    (1, 2): qt_items(1, 3) + opj_items(1, 1),
                    (1, 3): opj_items(2, 1),
                }

                def emit_filler_item(it):
                    kind = it[0]
                    if kind == "vp":
                        emit_vp_half(it[1], it[2])
                    elif kind == "qt":
                        emit_qt_d(it[1], it[2], it[3])
                    else:
                        emit_outproj(it[1], it[2], it[3])

                def emit_av(t, o_ps, p_ts, j):
                    for hp in range(2):
                        h = 2 * t + hp
                        nc.tensor.matmul(
                            o_ps[hp][:],
                            vp_sb[:, j * VW + h * 65:j * VW + h * 65 + 128],
                            p_ts[j][:, hp * 512:(hp + 1) * 512],
                            start=(j == 0), stop=(j == NJ - 1),
                        )

                for t in range(2):          # head pair
                    for qq in range(4):     # q quarter (512)
                        win = list(fillers[(t, qq)])
                        o_ps = {}
                        for hp in range(2):
                            o_ps[hp] = psB.tile(
                                [128, 512], F32, tag="o_ps", bufs=2,
                                name=f"o_ps{t}{qq}{hp}")
                        p_ts = {}
                        for j in range(NJ):
                            st = psB.tile([128, 1024], F32, tag="st",
                                          bufs=2, name=f"st{t}{qq}{j}")
                            for hp in range(2):
                                nc.tensor.matmul(
                                    st[:, hp * 512:(hp + 1) * 512],
                                    kt_sb[hp * 64:(hp + 1) * 64,
                                          t * SKV + j * 128:
                                          t * SKV + (j + 1) * 128],
                                    qt_sb[hp * 64:(hp + 1) * 64,
                                          t * SQ + qq * 512:
                                          t * SQ + (qq + 1) * 512],
                                    start=True, stop=True,
                                )
                            if win:
                                emit_filler_item(win.pop(0))
                            p_ts[j] = at.tile([128, 1024], BF16, tag="pt",
                                              bufs=8, name=f"pt{t}{qq}{j}")
                            nc.scalar.activation(p_ts[j][:], st[:],
                                                 AF.Exp, scale=scale)
                            # AV lags by two kv tiles: the PSUM accumulator
                            # handoff at block starts stays off the PE
                            # critical path
                            if j >= 2:
                                emit_av(t, o_ps, p_ts, j - 2)
                        for j in (NJ - 2, NJ - 1):
                            emit_av(t, o_ps, p_ts, j)
                        while win:
                            emit_filler_item(win.pop(0))
                        # normalize
                        o_half = o_sbA if qq < 2 else o_sbB
                        col = t * 1024 + (qq % 2) * 512
                        ot, rs = {}, {}
                        for hp in range(2):
                            ot[hp] = at.tile([64, 512], F32, tag="ot",
                                             bufs=4, name=f"ot{t}{qq}{hp}")
                            nc.vector.tensor_copy(ot[hp][:], o_ps[hp][0:64, :])
                            rs[hp] = at.tile([1, 512], F32, tag="rs", bufs=4,
                                             name=f"rs{t}{qq}{hp}")
                            nc.vector.tensor_copy(rs[hp][:], o_ps[hp][64:65, :])
                        for hp in range(2):
                            rcp = at.tile([1, 512], F32, tag="rcp",
                                          bufs=4, name=f"rcp{t}{qq}{hp}")
                            nc.vector.reciprocal_approx_fast(rcp[:], rs[hp][:])
                            bcs = at.tile([64, 512], F32, tag="bcs",
                                          bufs=4, name=f"bcs{t}{qq}{hp}")
                            nc.gpsimd.partition_broadcast(
                                bcs[:], rcp[:], channels=64)
                            nc.vector.tensor_tensor(
                                out=o_half[hp * 64:(hp + 1) * 64,
                                           col:col + 512],
                                in0=ot[hp][:], in1=bcs[:],
                                op=MUL)
                # tail: last quarter's second-half outproj
                for it in opj_items(3, 1):
                    emit_filler_item(it)

    nc.compile()
    return nc


def build_in_maps(inputs):
    query_input = np.asarray(inputs["query_input"], dtype=np.float32)
    kv_input = np.asarray(inputs["kv_input"], dtype=np.float32)
    Wq = np.asarray(inputs["Wq"], dtype=np.float32)
    bq = np.asarray(inputs["bq"], dtype=np.float32)
    Wkv = np.asarray(inputs["Wkv"], dtype=np.float32)
    bkv = np.asarray(inputs["bkv"], dtype=np.float32)
    Wo = np.asarray(inputs["Wo"], dtype=np.float32)

    Wk = Wkv[:, :D]
    Wv = Wkv[:, D:]
    bk = bkv[:D]

    xT = [np.ascontiguousarray(query_input[b].T).astype(BF16NP) for b in range(B)]
    kvT = [np.ascontiguousarray(kv_input[b].T).astype(BF16NP) for b in range(B)]

    in_maps = []
    for c in range(N_CORES):
        b, g = divmod(c, G)
        c0 = g * GC
        wvp = np.zeros((D, HPG * 65), np.float32)
        for h in range(HPG):
                wvp[:, h * 65:h * 65 + 64] = Wv[:, c0 + h * DH:c0 + (h + 1) * DH]
        bq2 = bq[c0:c0 + GC].reshape(2, 128).T.copy()
        bk2 = bk[c0:c0 + GC].reshape(2, 128).T.copy()
        in_maps.append({
                "xqT": xT[b],
                "xkvT": kvT[b],
                "wq": np.ascontiguousarray(Wq[:, c0:c0 + GC]).astype(BF16NP),
                "wk": np.ascontiguousarray(Wk[:, c0:c0 + GC]).astype(BF16NP),
                "wvp": wvp.astype(BF16NP),
                "wo": np.ascontiguousarray(Wo[c0:c0 + GC, :]).astype(BF16NP),
                "bq2": np.ascontiguousarray(bq2),
                "bk2": np.ascontiguousarray(bk2),
        })
    return in_maps


def kernel(query_input, kv_input, Wq, bq, Wkv, bkv, Wo, bo):
    global _nc_cache
    from concourse import bass_utils

    if _nc_cache is None:
        _nc_cache = _build_nc()
    nc = _nc_cache

    Wkv = np.asarray(Wkv, dtype=np.float32)
    Wo = np.asarray(Wo, dtype=np.float32)
    bo = np.asarray(bo, dtype=np.float32)
    bv = np.asarray(bkv, np.float32)[D:]

    in_maps = build_in_maps(dict(
        query_input=query_input, kv_input=kv_input, Wq=Wq, bq=bq,
        Wkv=Wkv, bkv=bkv, Wo=Wo))

    res = bass_utils.run_bass_kernel_spmd(nc, in_maps,
                                          core_ids=list(range(N_CORES)))

    # gather: sum the 8 head-pair partials per batch; add biases the device
    # left out (bo, and bv which passes through Wo since softmax rows sum to 1)
    tail = bv @ Wo + bo
    out = np.empty((B, SQ, D), np.float32)
    for b in range(B):
        acc = res.results[b * G + 0]["out_p0"].astype(np.float32).copy()
        acc += res.results[b * G + 0]["out_p1"]
        for g in range(1, G):
                acc += res.results[b * G + g]["out_p0"]
                acc += res.results[b * G + g]["out_p1"]
        out[b] = acc + tail[None, :]
    return out
